# revision 1
# baseline (speedup 1.0000x reference)
"""GCN 2-layer message-passing kernel for Trainium2 (8 NeuronCores, Bass/Tile).

Strategy (graph/data parallel per the sharding hint):
  - Nodes partitioned into 8 contiguous ranges (6250 per core).
  - Host does INTEGER/index prep only: add self-loops, bucket edges by
    destination core/block-of-128, sort by source, build gather-index +
    dst-slot metadata, integer in-degree counts. All floating-point math
    runs on device.
  - Per layer: each core computes g = dinv * (x @ W) rows for its own
    nodes (PE matmul + ACT per-partition scale, cast to bf16), AllGather
    of the bf16 g-table across the 8 cores (halo exchange). Then, per
    chunk of destination blocks, one dma_gather instruction fetches all
    message rows (g[src]) for the chunk's edge tiles; per 128-edge tile a
    0/1 selection matrix (DVE is_equal against an iota matrix) scatter-
    reduces the messages into the block's PSUM accumulator via one PE
    matmul; the block output is
      relu(dinv * segsum + x @ res_w + conv_b + res_b)
    with bias via a K=1 outer-product matmul and residual accumulated in
    a second PSUM bank; dst-degree scaling via ACT per-partition scale.
  - dma_gather uses int16 indices, so the gather table is addressed in a
    low half (rows < 32768) and a high half; each block's edges are
    split into low/high tile groups (host-side integer split).

kernel(**inputs) takes FULL inputs and returns the FULL [50000, 128]
float32 output.
"""
import sys
from contextlib import ExitStack

import numpy as np

if '/opt/trn_rl_repo' not in sys.path:
    sys.path.insert(0, '/opt/trn_rl_repo')

import ml_dtypes

from concourse import bacc, mybir, tile
from concourse.bass_utils import run_bass_kernel_spmd
from concourse.vector_clock import ScopedClock


def _patched_drain_and_barrier(self, tick_clock, wait_clock):
    """Split the kernel-tail drain's sem waits across single-wait drains:
    walrus's NO_STRUCT codegen rejects >1 sync wait on InstDrain."""
    drain_inst = self.nc.sync.drain()
    wait_clock.add_sem_waits(drain_inst.ins,
                             ScopedClock({None: tick_clock.global_clock}))
    si = drain_inst.ins.sync_info
    if si is not None and si.on_wait is not None and len(si.on_wait) > 1:
        waits = list(si.on_wait)
        del si.on_wait[1:]
        for w in waits[1:]:
            d2 = self.nc.sync.drain()
            si2 = d2.ins.sync_info
            if si2 is None:
                d2.ins.sync_info = mybir.SyncInfo(on_wait=[w], on_update=[])
            else:
                si2.on_wait.append(w)
    self.nc.all_engine_barrier()
    assert self.sems is not None
    popped = self.nc._tile_sem_poison_stack.pop()
    assert popped is self._sem_poison
    self.nc.clear_and_free_semaphores(list(self.sems.allocated().values()))
    self.nc.all_engine_barrier()


tile.TileContext._drain_and_barrier = _patched_drain_and_barrier


def split_sync_waits(nc, max_waits=1):
    """Walrus codegen rejects >1 sync wait on several instruction encodings.
    Hoist excess waits onto same-engine no-ops placed just before."""
    import bass_rust
    try:
        funcs = list(nc.m.functions)
    except Exception:
        funcs = [nc.main_func]
    seen = 0
    for fn in funcs:
        for bb in fn.blocks:
            insts = bb.instructions
            new = []
            for ins in insts:
                si = ins.sync_info
                if si is not None and si.on_wait and len(si.on_wait) > max_waits:
                    waits = list(si.on_wait)
                    extra, keep = waits[:-max_waits], waits[-max_waits:]
                    for w in extra:
                        nop = bass_rust.InstNoOp(
                            name=f"I-waitsplit-{seen}", ins=[], outs=[])
                        seen += 1
                        nop.engine = ins.engine
                        nop.sync_info = mybir.SyncInfo(on_wait=[w], on_update=[])
                        new.append(nop)
                    del si.on_wait[:]
                    si.on_wait.extend(keep)
                new.append(ins)
            insts[:] = new
    return seen


bf16 = ml_dtypes.bfloat16
P = 128          # partitions / tile edge
C = 8            # cores
D = 128          # hidden dim
HI = 32768       # int16 index reach of dma_gather
CB = 4           # dst blocks per gather chunk


# ---------------------------------------------------------------------------
# Host-side integer/index prep (sharding + metadata; no FP math on values)
# ---------------------------------------------------------------------------

def prep(edge_index, n_nodes):
    N = n_nodes
    npc = N // C
    assert npc * C == N
    B = (npc + P - 1) // P
    npad = B * P

    ei = np.asarray(edge_index)
    # self-loops are handled on-device via an identity matmul per block;
    # they still count toward the degree.
    src_all = ei[0].astype(np.int64)
    dst_all = ei[1].astype(np.int64)
    deg_all = np.bincount(dst_all, minlength=N) + 1

    own_s = src_all // npc
    row_all = own_s * npad + (src_all - own_s * npc)   # row in concat g table

    owner_all = dst_all // npc
    per_core = []
    nlo = np.zeros((C, B), dtype=np.int64)
    nhi = np.zeros((C, B), dtype=np.int64)
    for c in range(C):
        m = owner_all == c
        r = row_all[m]
        dloc = dst_all[m] - c * npc
        blk = dloc >> 7
        slot = dloc & 127
        hi = (r >= HI).astype(np.int64)
        # sort by (block, hi, row) so each block = [lo edges..., hi edges...]
        order = np.lexsort((r, hi, blk))
        r, blk, slot, hi = r[order], blk[order], slot[order], hi[order]
        per_core.append((r, blk, slot))
        for b in range(B):
            mb = blk == b
            nhi[c, b] = (hi[mb]).sum()
            nlo[c, b] = mb.sum() - nhi[c, b]

    T_lo = np.maximum((nlo.max(axis=0) + P - 1) // P, 1)
    T_hi = (nhi.max(axis=0) + P - 1) // P            # may be 0 for a block
    T_b = (T_lo + T_hi).astype(np.int64)
    T_total = int(T_b.sum())

    # per-block tile layout: T_lo[b] low tiles then T_hi[b] high tiles
    tile_base = np.concatenate([[0], np.cumsum(T_b)])
    # low/high gather index sequences (tile-major, per core)
    n_lo_total = int(T_lo.sum()) * P
    n_hi_total = int(T_hi.sum()) * P
    lo_base = np.concatenate([[0], np.cumsum(T_lo)])   # in tiles
    hi_base = np.concatenate([[0], np.cumsum(T_hi)])

    slots = np.full((C, T_total * P), -1.0, dtype=np.float32)
    idx_lo = np.zeros((C, n_lo_total), dtype=np.int64)
    idx_hi = np.zeros((C, max(n_hi_total, 16)), dtype=np.int64)
    for c in range(C):
        r, blk, slot = per_core[c]
        bstart = np.concatenate([[0], np.cumsum(nlo[c] + nhi[c])])
        for b in range(B):
            e0, e1 = bstart[b], bstart[b + 1]
            k_lo = int(nlo[c, b])
            rl, sl = r[e0:e0 + k_lo], slot[e0:e0 + k_lo]
            rh, sh = r[e0 + k_lo:e1], slot[e0 + k_lo:e1]
            o = lo_base[b] * P
            idx_lo[c, o:o + k_lo] = rl
            o = hi_base[b] * P
            idx_hi[c, o:o + len(rh)] = rh - HI
            o = tile_base[b] * P
            slots[c, o:o + k_lo] = sl
            o2 = (tile_base[b] + T_lo[b]) * P
            slots[c, o2:o2 + len(sh)] = sh

    deg = np.ones((C, P, B), dtype=np.float32)
    for c in range(C):
        dpad = np.ones(npad, dtype=np.float32)
        dpad[:npc] = deg_all[c * npc:(c + 1) * npc].astype(np.float32)
        deg[c] = dpad.reshape(B, P).T

    def pack16(a):
        # wrapped layout: element j -> [j % 16, j // 16], replicated to the
        # 8 Q7 cores' partition groups (128 partitions total)
        n = a.shape[1]
        w = a.reshape(a.shape[0], n // 16, 16).transpose(0, 2, 1).astype(np.int16)
        return np.tile(w, (1, 8, 1)).copy()

    # chunking of blocks for gather calls
    chunks = []
    for b0 in range(0, B, CB):
        b1 = min(b0 + CB, B)
        chunks.append(dict(
            b0=b0, b1=b1,
            lo_t0=int(lo_base[b0]), lo_t1=int(lo_base[b1]),
            hi_t0=int(hi_base[b0]), hi_t1=int(hi_base[b1]),
        ))

    return dict(
        npc=npc, npad=npad, B=B,
        T_lo=T_lo.tolist(), T_hi=T_hi.tolist(), T_b=T_b.tolist(),
        tile_base=tile_base.tolist(), T_total=T_total,
        n_lo16=n_lo_total // 16, n_hi16=max(n_hi_total, 16) // 16,
        chunks=chunks,
        idx_lo=pack16(idx_lo), idx_hi=pack16(idx_hi),
        slots=slots.reshape(C, T_total, P).transpose(0, 2, 1).copy(),
        deg=deg,
    )


# ---------------------------------------------------------------------------
# Device program (uniform across the 8 cores)
# ---------------------------------------------------------------------------

def build_program(meta):
    npad, B, T_total = meta['npad'], meta['B'], meta['T_total']
    T_lo, T_hi, tile_base = meta['T_lo'], meta['T_hi'], meta['tile_base']
    chunks = meta['chunks']
    TBL = C * npad
    f32 = mybir.dt.float32
    bf = mybir.dt.bfloat16
    max_lo_tiles = max(ch['lo_t1'] - ch['lo_t0'] for ch in chunks)
    max_hi_tiles = max(ch['hi_t1'] - ch['hi_t0'] for ch in chunks)

    nc = bacc.Bacc(None, target_bir_lowering=False)
    xT_p = nc.declare_dram_parameter("xT", [P, npad], f32, isOutput=False)
    w1_p = nc.declare_dram_parameter("w1", [P, D], f32, isOutput=False)
    w2_p = nc.declare_dram_parameter("w2", [P, D], f32, isOutput=False)
    rw_p = nc.declare_dram_parameter("resw", [P, D], f32, isOutput=False)
    cb_p = nc.declare_dram_parameter("convb", [2, D], f32, isOutput=False)
    rb_p = nc.declare_dram_parameter("resb", [1, D], f32, isOutput=False)
    deg_p = nc.declare_dram_parameter("deg", [P, B], f32, isOutput=False)
    ilo_p = nc.declare_dram_parameter("idx_lo", [128, meta['n_lo16']], mybir.dt.int16, isOutput=False)
    ihi_p = nc.declare_dram_parameter("idx_hi", [128, meta['n_hi16']], mybir.dt.int16, isOutput=False)
    slot_p = nc.declare_dram_parameter("slot", [P, T_total], bf, isOutput=False)
    iota_p = nc.declare_dram_parameter("iota", [P, P], bf, isOutput=False)
    ident_p = nc.declare_dram_parameter("ident", [P, P], f32, isOutput=False)
    ones_p = nc.declare_dram_parameter("ones", [1, D], bf, isOutput=False)
    out_p = nc.declare_dram_parameter("out", [npad, D], f32, isOutput=True)

    g1_own = nc.dram_tensor("g1_own", [npad, D], bf)
    g2_own = nc.dram_tensor("g2_own", [npad, D], bf)
    g1_full = nc.dram_tensor("g1_full", [TBL, D], bf, addr_space="Shared")
    g2_full = nc.dram_tensor("g2_full", [TBL, D], bf, addr_space="Shared")

    with tile.TileContext(nc) as tc, ExitStack() as ctx:
        const = ctx.enter_context(tc.tile_pool(name="const", bufs=1))
        gbuf = ctx.enter_context(tc.tile_pool(name="gbuf", bufs=2))
        work = ctx.enter_context(tc.tile_pool(name="work", bufs=6))
        outp = ctx.enter_context(tc.tile_pool(name="outp", bufs=3))
        psum = ctx.enter_context(tc.tile_pool(name="psum", bufs=2, space="PSUM"))

        # ---- constants / persistent state ----
        xT = const.tile([P, npad], f32)
        nc.sync.dma_start(out=xT[:], in_=xT_p[:, :])
        x1T = const.tile([P, npad], bf)          # layer-1 output, transposed
        w1 = const.tile([P, D], f32)
        nc.sync.dma_start(out=w1[:], in_=w1_p[:, :])
        w2f = const.tile([P, D], f32)
        nc.sync.dma_start(out=w2f[:], in_=w2_p[:, :])
        rwf = const.tile([P, D], f32)
        nc.sync.dma_start(out=rwf[:], in_=rw_p[:, :])
        w2b = const.tile([P, D], bf)
        nc.vector.tensor_copy(out=w2b[:], in_=w2f[:])
        rwb = const.tile([P, D], bf)
        nc.vector.tensor_copy(out=rwb[:], in_=rwf[:])

        rb = const.tile([1, D], f32)
        nc.sync.dma_start(out=rb[:], in_=rb_p[:, :])
        bcomb = []
        for l in range(2):
            cbl = const.tile([1, D], f32, tag=f"cb{l}")
            nc.sync.dma_start(out=cbl[:], in_=cb_p[l:l + 1, :])
            bc = const.tile([1, D], bf, tag=f"bcomb{l}")
            nc.vector.tensor_tensor(out=bc[:], in0=cbl[:], in1=rb[:],
                                    op=mybir.AluOpType.add)
            bcomb.append(bc)
        ones1 = const.tile([1, D], bf)
        nc.sync.dma_start(out=ones1[:], in_=ones_p[:, :])

        iota = const.tile([P, P], bf)
        nc.sync.dma_start(out=iota[:], in_=iota_p[:, :])
        ident = const.tile([P, P], f32)
        nc.sync.dma_start(out=ident[:], in_=ident_p[:, :])
        ident_bf = const.tile([P, P], bf)
        nc.vector.tensor_copy(out=ident_bf[:], in_=ident[:])

        ilo = const.tile([128, meta['n_lo16']], mybir.dt.int16)
        nc.sync.dma_start(out=ilo[:], in_=ilo_p[:, :])
        ihi = const.tile([128, meta['n_hi16']], mybir.dt.int16)
        nc.sync.dma_start(out=ihi[:], in_=ihi_p[:, :])
        slots = const.tile([P, T_total], bf)
        nc.sync.dma_start(out=slots[:], in_=slot_p[:, :])

        degt = const.tile([P, B], f32)
        nc.sync.dma_start(out=degt[:], in_=deg_p[:, :])
        sdeg = const.tile([P, B], f32)
        nc.scalar.activation(out=sdeg[:], in_=degt[:],
                             func=mybir.ActivationFunctionType.Sqrt)
        dinv = const.tile([P, B], f32)
        nc.vector.reciprocal(out=dinv[:], in_=sdeg[:])

        # ---- phase 1: g1 = dinv * (x @ W1) for own rows, then AllGather ----
        with nc.named_scope("phase1"):
            for b in range(B):
                cs = slice(b * P, (b + 1) * P)
                ph = psum.tile([P, D], f32, tag="ph")
                nc.tensor.matmul(out=ph[:], lhsT=xT[:, cs], rhs=w1[:],
                                 start=True, stop=True)
                gb = outp.tile([P, D], bf, tag="gb")
                nc.scalar.activation(out=gb[:], in_=ph[:],
                                     func=mybir.ActivationFunctionType.Copy,
                                     scale=dinv[:, b:b + 1])
                nc.sync.dma_start(out=g1_own[cs, :], in_=gb[:])
        with nc.named_scope("ag1"):
            nc.gpsimd.collective_compute(
                "AllGather", mybir.AluOpType.bypass,
                replica_groups=[list(range(C))],
                ins=[g1_own[:, :]], outs=[g1_full[:, :]])

        def layer(l, g_full, g_own_l):
            selmax = max(T_lo[b] + T_hi[b] for b in range(B))
            for ch in chunks:
                nlo_t = ch['lo_t1'] - ch['lo_t0']
                nhi_t = ch['hi_t1'] - ch['hi_t0']
                glo = gbuf.tile([P, max_lo_tiles, D], bf, tag="glo")
                nc.gpsimd.dma_gather(
                    out_ap=glo[:, :nlo_t, :], in_ap=g_full[:, :],
                    idxs_ap=ilo[:, ch['lo_t0'] * 8:ch['lo_t1'] * 8],
                    num_idxs=nlo_t * P, num_idxs_reg=nlo_t * P, elem_size=D,
                    single_packet=False)
                if nhi_t > 0:
                    ghi = gbuf.tile([P, max(max_hi_tiles, 1), D], bf, tag="ghi")
                    nc.gpsimd.dma_gather(
                        out_ap=ghi[:, :nhi_t, :], in_ap=g_full[HI:, :],
                        idxs_ap=ihi[:, ch['hi_t0'] * 8:ch['hi_t1'] * 8],
                        num_idxs=nhi_t * P, num_idxs_reg=nhi_t * P, elem_size=D,
                        single_packet=False)
                for b in range(ch['b0'], ch['b1']):
                    cs = slice(b * P, (b + 1) * P)
                    pB = psum.tile([P, D], f32, tag="pB")
                    nc.tensor.matmul(out=pB[:], lhsT=ones1[:], rhs=bcomb[l][:],
                                     start=True, stop=False)
                    if l == 0:
                        nc.tensor.matmul(out=pB[:], lhsT=xT[:, cs], rhs=rwf[:],
                                         start=False, stop=True)
                    else:
                        nc.tensor.matmul(out=pB[:], lhsT=x1T[:, cs], rhs=rwb[:],
                                         start=False, stop=True)
                    pA = psum.tile([P, D], f32, tag="pA")
                    nt = T_lo[b] + T_hi[b]
                    lo_off = sum(T_lo[ch['b0']:b])
                    hi_off = sum(T_hi[ch['b0']:b])
                    tb = tile_base[b]
                    # all sel matrices of the block in one DVE op
                    selb = work.tile([P, selmax, P], bf, tag="sel")
                    nc.vector.tensor_tensor(
                        out=selb[:, :nt, :],
                        in0=slots[:, tb:tb + nt]
                            .rearrange("p (k o) -> p k o", o=1)
                            .to_broadcast([P, nt, P]),
                        in1=iota[:].rearrange("p (o d) -> p o d", o=1)
                            .to_broadcast([P, nt, P]),
                        op=mybir.AluOpType.is_equal)
                    # self-loop contribution: psum += I @ g_own[block]
                    gsb = work.tile([P, D], bf, tag="gsb")
                    nc.sync.dma_start(out=gsb[:], in_=g_own_l[cs, :])
                    nc.tensor.matmul(out=pA[:], lhsT=ident_bf[:], rhs=gsb[:],
                                     start=True, stop=False)
                    for t in range(nt):
                        if t < T_lo[b]:
                            src = glo[:, lo_off + t, :]
                        else:
                            src = ghi[:, hi_off + (t - T_lo[b]), :]
                        nc.tensor.matmul(out=pA[:], lhsT=selb[:, t, :], rhs=src,
                                         start=False, stop=(t == nt - 1))
                    t1 = outp.tile([P, D], f32, tag="t1")
                    nc.scalar.activation(out=t1[:], in_=pA[:],
                                         func=mybir.ActivationFunctionType.Copy,
                                         scale=dinv[:, b:b + 1])
                    t2 = outp.tile([P, D], f32, tag="t2")
                    nc.vector.tensor_tensor(out=t2[:], in0=t1[:], in1=pB[:],
                                            op=mybir.AluOpType.add)
                    xo = outp.tile([P, D], f32, tag="xo")
                    nc.scalar.activation(out=xo[:], in_=t2[:],
                                         func=mybir.ActivationFunctionType.Relu)
                    if l == 0:
                        pT = psum.tile([P, D], f32, tag="pT")
                        nc.tensor.transpose(out=pT[:], in_=xo[:], identity=ident[:])
                        nc.vector.tensor_copy(out=x1T[:, cs], in_=pT[:])
                        ph2 = psum.tile([P, D], f32, tag="ph")
                        nc.tensor.matmul(out=ph2[:], lhsT=x1T[:, cs], rhs=w2b[:],
                                         start=True, stop=True)
                        g2b = outp.tile([P, D], bf, tag="gb")
                        nc.scalar.activation(out=g2b[:], in_=ph2[:],
                                             func=mybir.ActivationFunctionType.Copy,
                                             scale=dinv[:, b:b + 1])
                        nc.sync.dma_start(out=g2_own[cs, :], in_=g2b[:])
                    else:
                        nc.sync.dma_start(out=out_p[cs, :], in_=xo[:])

        with nc.named_scope("layer1"):
            layer(0, g1_full, g1_own)
        with nc.named_scope("ag2"):
            nc.gpsimd.collective_compute(
                "AllGather", mybir.AluOpType.bypass,
                replica_groups=[list(range(C))],
                ins=[g2_own[:, :]], outs=[g2_full[:, :]])
        with nc.named_scope("layer2"):
            layer(1, g2_full, g2_own)
    return nc


# ---------------------------------------------------------------------------
# Entry point
# ---------------------------------------------------------------------------

def make_inputs(x, conv_w, conv_b, res_w, res_b, meta):
    npc, npad = meta['npc'], meta['npad']
    iota = np.tile(np.arange(P, dtype=np.float32), (P, 1)).astype(bf16)
    in_maps = []
    for c in range(C):
        xT = np.zeros((P, npad), dtype=np.float32)
        xT[:, :npc] = np.asarray(x[c * npc:(c + 1) * npc], dtype=np.float32).T
        in_maps.append({
            "xT": xT,
            "w1": np.asarray(conv_w[0], dtype=np.float32),
            "w2": np.asarray(conv_w[1], dtype=np.float32),
            "resw": np.asarray(res_w, dtype=np.float32),
            "convb": np.asarray(conv_b, dtype=np.float32),
            "resb": np.asarray(res_b, dtype=np.float32).reshape(1, D),
            "deg": meta['deg'][c],
            "idx_lo": meta['idx_lo'][c],
            "idx_hi": meta['idx_hi'][c],
            "slot": meta['slots'][c].astype(bf16),
            "iota": iota,
            "ident": np.eye(P, dtype=np.float32),
            "ones": np.ones((1, D), dtype=np.float32).astype(bf16),
        })
    return in_maps


def run(x, edge_index, conv_w, conv_b, res_w, res_b, trace=False, trace_kwargs=None):
    N = x.shape[0]
    meta = prep(edge_index, N)
    nc = build_program(meta)
    nc.compile()
    split_sync_waits(nc)
    in_maps = make_inputs(x, conv_w, conv_b, res_w, res_b, meta)
    res = run_bass_kernel_spmd(nc, in_maps, list(range(C)), trace=trace,
                               **(trace_kwargs or {}))
    npc = meta['npc']
    out = np.concatenate([np.asarray(res.results[c]["out"])[:npc]
                          for c in range(C)], axis=0)
    return out.astype(np.float32), res


def kernel(x, edge_index, conv_w, conv_b, res_w, res_b):
    out, _ = run(x, edge_index, conv_w, conv_b, res_w, res_b, trace=False)
    return out



# revision 12
# speedup vs baseline: 1.0482x; 1.0482x over previous
"""GCN 2-layer message-passing kernel for Trainium2 (8 NeuronCores, Bass/Tile).

Strategy (graph/data parallel):
  - Nodes partitioned into 8 contiguous ranges (6250 per core, padded 6272).
  - Host does INTEGER/index prep only: bucket edges by (dst core, dst
    block), split by source-row class, sort, build gather-index + dst-slot
    metadata, integer in-degree counts. All FP math runs on device.
  - The halo table g = dinv * (x @ W) is split into TWO source-row CLASSES
    (local rows [0,3072) and [3072,6272)); each class is AllGathered
    separately so the class-0 table is available while the tail of the
    producing phase still runs. This keeps the GpSimd engine (whose
    software descriptor generation for dma_gather is the kernel's hard
    bottleneck at ~8ns/row) busy continuously across layer boundaries.
  - Per layer, per chunk of 7 destination blocks: one dma_gather per class
    fetches all message rows; per 128-edge tile a 0/1 selection matrix
    (DVE is_equal vs iota) scatter-reduces messages into the block's PSUM
    accumulator via one PE matmul; self-loops via an identity matmul;
    bias+residual accumulate in a second PSUM bank; dst-degree scaling via
    ACT per-partition scale. Layer-2 g-table rows are produced inside the
    layer-1 block loop and AllGathered class-by-class as soon as ready.
  - A tiny warm-up dma_gather at t=0 absorbs the Q7 ucode first-call cost.

kernel(**inputs) takes FULL inputs and returns the FULL [50000, 128]
float32 output.
"""
import sys
from contextlib import ExitStack

import numpy as np

if '/opt/trn_rl_repo' not in sys.path:
    sys.path.insert(0, '/opt/trn_rl_repo')

import ml_dtypes

from concourse import bacc, mybir, tile
from concourse.bass_utils import run_bass_kernel_spmd
from concourse.vector_clock import ScopedClock


def _patched_drain_and_barrier(self, tick_clock, wait_clock):
    """Split the kernel-tail drain's sem waits across single-wait drains:
    walrus's NO_STRUCT codegen rejects >1 sync wait on InstDrain."""
    drain_inst = self.nc.sync.drain()
    wait_clock.add_sem_waits(drain_inst.ins,
                             ScopedClock({None: tick_clock.global_clock}))
    si = drain_inst.ins.sync_info
    if si is not None and si.on_wait is not None and len(si.on_wait) > 1:
        waits = list(si.on_wait)
        del si.on_wait[1:]
        for w in waits[1:]:
            d2 = self.nc.sync.drain()
            si2 = d2.ins.sync_info
            if si2 is None:
                d2.ins.sync_info = mybir.SyncInfo(on_wait=[w], on_update=[])
            else:
                si2.on_wait.append(w)
    self.nc.all_engine_barrier()
    assert self.sems is not None
    popped = self.nc._tile_sem_poison_stack.pop()
    assert popped is self._sem_poison
    self.nc.clear_and_free_semaphores(list(self.sems.allocated().values()))
    self.nc.all_engine_barrier()


tile.TileContext._drain_and_barrier = _patched_drain_and_barrier


def split_sync_waits(nc, max_waits=1):
    """Walrus codegen rejects >1 sync wait on several instruction encodings.
    Hoist excess waits onto same-engine no-ops placed just before."""
    import bass_rust
    try:
        funcs = list(nc.m.functions)
    except Exception:
        funcs = [nc.main_func]
    seen = 0
    for fn in funcs:
        for bb in fn.blocks:
            insts = bb.instructions
            new = []
            for ins in insts:
                si = ins.sync_info
                if si is not None and si.on_wait and len(si.on_wait) > max_waits:
                    waits = list(si.on_wait)
                    extra, keep = waits[:-max_waits], waits[-max_waits:]
                    for w in extra:
                        nop = bass_rust.InstNoOp(
                            name=f"I-waitsplit-{seen}", ins=[], outs=[])
                        seen += 1
                        nop.engine = ins.engine
                        nop.sync_info = mybir.SyncInfo(on_wait=[w], on_update=[])
                        new.append(nop)
                    del si.on_wait[:]
                    si.on_wait.extend(keep)
                new.append(ins)
            insts[:] = new
    return seen


bf16 = ml_dtypes.bfloat16
P = 128          # partitions / tile edge
C = 8            # cores
D = 128          # hidden dim
NCLS = 2         # source-row classes
CLS_BLK = (32, 17)           # blocks per class (32*128=4096, 17*128=2176)
CLS_BASE = (0, 4096)
CLS_SZ = (4096, 2176)        # class-0 table = 8*4096 = 32768 rows (int16 max)
CB = 5           # dst blocks per gather chunk


# ---------------------------------------------------------------------------
# Host-side integer/index prep (sharding + metadata; no FP math on values)
# ---------------------------------------------------------------------------

def prep(edge_index, n_nodes):
    N = n_nodes
    npc = N // C
    assert npc * C == N
    B = (npc + P - 1) // P
    npad = B * P
    assert B == CLS_BLK[0] + CLS_BLK[1] and npad == CLS_SZ[0] + CLS_SZ[1]

    ei = np.asarray(edge_index)
    src_all = ei[0].astype(np.int64)
    dst_all = ei[1].astype(np.int64)
    # self-loops handled on-device via identity matmul; count in degree
    deg_all = np.bincount(dst_all, minlength=N) + 1

    own_s = src_all // npc
    loc_s = src_all - own_s * npc
    cls_all = (loc_s >= CLS_SZ[0]).astype(np.int64)
    row_all = np.where(cls_all == 0,
                       own_s * CLS_SZ[0] + loc_s,
                       own_s * CLS_SZ[1] + (loc_s - CLS_BASE[1]))

    owner_all = dst_all // npc
    per_core = []
    cnt = np.zeros((C, NCLS, B), dtype=np.int64)
    for c in range(C):
        m = owner_all == c
        r = row_all[m]
        k = cls_all[m]
        dloc = dst_all[m] - c * npc
        blk = dloc >> 7
        slot = dloc & 127
        order = np.lexsort((r, blk, k))
        r, k, blk, slot = r[order], k[order], blk[order], slot[order]
        per_core.append((r, k, blk, slot))
        for kk in range(NCLS):
            mk = k == kk
            cnt[c, kk] = np.bincount(blk[mk], minlength=B)

    # uniform tile counts: max over cores, per (class, block)
    T = [np.ceil(cnt[:, kk, :].max(axis=0) / P).astype(np.int64)
         for kk in range(NCLS)]
    tile_base = [np.concatenate([[0], np.cumsum(T[kk])]) for kk in range(NCLS)]
    T_total = [int(T[kk].sum()) for kk in range(NCLS)]

    idx = [np.zeros((C, T_total[kk] * P), dtype=np.int64) for kk in range(NCLS)]
    slots = [np.full((C, T_total[kk] * P), -1.0, dtype=np.float32)
             for kk in range(NCLS)]
    for c in range(C):
        r, k, blk, slot = per_core[c]
        for kk in range(NCLS):
            mk = k == kk
            rk, bk, sk = r[mk], blk[mk], slot[mk]
            bstart = np.concatenate([[0], np.cumsum(np.bincount(bk, minlength=B))])
            for b in range(B):
                e0, e1 = bstart[b], bstart[b + 1]
                o = tile_base[kk][b] * P
                idx[kk][c, o:o + (e1 - e0)] = rk[e0:e1]
                slots[kk][c, o:o + (e1 - e0)] = sk[e0:e1]

    deg = np.ones((C, P, B), dtype=np.float32)
    for c in range(C):
        dpad = np.ones(npad, dtype=np.float32)
        dpad[:npc] = deg_all[c * npc:(c + 1) * npc].astype(np.float32)
        deg[c] = dpad.reshape(B, P).T

    def pack16(a):
        # wrapped layout: element j -> [j % 16, j // 16], replicated to the
        # 8 Q7 cores' partition groups (128 partitions total)
        n = a.shape[1]
        w = a.reshape(a.shape[0], n // 16, 16).transpose(0, 2, 1).astype(np.int16)
        return np.tile(w, (1, 8, 1)).copy()

    chunks = []
    for b0 in range(0, B, CB):
        b1 = min(b0 + CB, B)
        chunks.append(dict(
            b0=b0, b1=b1,
            t0=[int(tile_base[kk][b0]) for kk in range(NCLS)],
            t1=[int(tile_base[kk][b1]) for kk in range(NCLS)],
        ))

    return dict(
        npc=npc, npad=npad, B=B,
        T=[T[kk].tolist() for kk in range(NCLS)],
        tile_base=[tile_base[kk].tolist() for kk in range(NCLS)],
        T_total=T_total, chunks=chunks,
        idx=[pack16(idx[kk]) for kk in range(NCLS)],
        slots=[slots[kk].reshape(C, T_total[kk], P).transpose(0, 2, 1).copy()
               for kk in range(NCLS)],
        deg=deg,
    )


# ---------------------------------------------------------------------------
# Device program (uniform across the 8 cores)
# ---------------------------------------------------------------------------

def build_program(meta):
    npad, B = meta['npad'], meta['B']
    T, tile_base, T_total = meta['T'], meta['tile_base'], meta['T_total']
    chunks = meta['chunks']
    f32 = mybir.dt.float32
    bf = mybir.dt.bfloat16
    max_ct = [max(ch['t1'][kk] - ch['t0'][kk] for ch in chunks)
              for kk in range(NCLS)]
    selmax = [max(T[kk]) for kk in range(NCLS)]

    nc = bacc.Bacc(None, target_bir_lowering=False)
    xT_p = nc.declare_dram_parameter("xT", [P, npad], f32, isOutput=False)
    w1_p = nc.declare_dram_parameter("w1", [P, D], f32, isOutput=False)
    w2_p = nc.declare_dram_parameter("w2", [P, D], f32, isOutput=False)
    rw_p = nc.declare_dram_parameter("resw", [P, D], f32, isOutput=False)
    cb_p = nc.declare_dram_parameter("convb", [2, D], f32, isOutput=False)
    rb_p = nc.declare_dram_parameter("resb", [1, D], f32, isOutput=False)
    deg_p = nc.declare_dram_parameter("deg", [P, B], f32, isOutput=False)
    idx_p = [nc.declare_dram_parameter(f"idx{kk}", [128, T_total[kk] * 8],
                                       mybir.dt.int16, isOutput=False)
             for kk in range(NCLS)]
    slot_p = [nc.declare_dram_parameter(f"slot{kk}", [P, T_total[kk]], bf,
                                        isOutput=False)
              for kk in range(NCLS)]
    warm_p = nc.declare_dram_parameter("warmidx", [128, 8], mybir.dt.int16,
                                       isOutput=False)
    iota_p = nc.declare_dram_parameter("iota", [P, P], bf, isOutput=False)
    ident_p = nc.declare_dram_parameter("ident", [P, P], f32, isOutput=False)
    ones_p = nc.declare_dram_parameter("ones", [1, D], bf, isOutput=False)
    out_p = nc.declare_dram_parameter("out", [npad, D], f32, isOutput=True)

    g_own = [[nc.dram_tensor(f"g{l}o{kk}", [CLS_SZ[kk], D], bf)
              for kk in range(NCLS)] for l in range(2)]
    g_full = [[nc.dram_tensor(f"g{l}f{kk}", [C * CLS_SZ[kk], D], bf,
                              addr_space="Shared")
               for kk in range(NCLS)] for l in range(2)]

    with tile.TileContext(nc) as tc, ExitStack() as ctx:
        const = ctx.enter_context(tc.tile_pool(name="const", bufs=1))
        gbuf = ctx.enter_context(tc.tile_pool(name="gbuf", bufs=3))
        work = ctx.enter_context(tc.tile_pool(name="work", bufs=6))
        outp = ctx.enter_context(tc.tile_pool(name="outp", bufs=3))
        psum = ctx.enter_context(tc.tile_pool(name="psum", bufs=2, space="PSUM"))

        # ---- warm-up gather: absorb Q7 ucode first-call cost immediately
        warmidx = const.tile([128, 8], mybir.dt.int16)
        nc.sync.dma_start(out=warmidx[:], in_=warm_p[:, :])
        warmg = const.tile([P, 1, D], bf)
        nc.gpsimd.dma_gather(out_ap=warmg[:], in_ap=g_own[0][0][:, :],
                             idxs_ap=warmidx[:], num_idxs=128,
                             num_idxs_reg=128, elem_size=D,
                             single_packet=False)

        # ---- constants / persistent state ----
        xT = const.tile([P, npad], f32)
        nc.sync.dma_start(out=xT[:], in_=xT_p[:, :])
        x1T = const.tile([P, npad], bf)          # layer-1 output, transposed
        w1 = const.tile([P, D], f32)
        nc.sync.dma_start(out=w1[:], in_=w1_p[:, :])
        w2f = const.tile([P, D], f32)
        nc.sync.dma_start(out=w2f[:], in_=w2_p[:, :])
        rwf = const.tile([P, D], f32)
        nc.sync.dma_start(out=rwf[:], in_=rw_p[:, :])
        w2b = const.tile([P, D], bf)
        nc.vector.tensor_copy(out=w2b[:], in_=w2f[:])
        rwb = const.tile([P, D], bf)
        nc.vector.tensor_copy(out=rwb[:], in_=rwf[:])

        rb = const.tile([1, D], f32)
        nc.sync.dma_start(out=rb[:], in_=rb_p[:, :])
        bcomb = []
        for l in range(2):
            cbl = const.tile([1, D], f32, tag=f"cb{l}")
            nc.sync.dma_start(out=cbl[:], in_=cb_p[l:l + 1, :])
            bc = const.tile([1, D], bf, tag=f"bcomb{l}")
            nc.vector.tensor_tensor(out=bc[:], in0=cbl[:], in1=rb[:],
                                    op=mybir.AluOpType.add)
            bcomb.append(bc)
        ones1 = const.tile([1, D], bf)
        nc.sync.dma_start(out=ones1[:], in_=ones_p[:, :])

        iota = const.tile([P, P], bf)
        nc.sync.dma_start(out=iota[:], in_=iota_p[:, :])
        ident = const.tile([P, P], f32)
        nc.sync.dma_start(out=ident[:], in_=ident_p[:, :])
        ident_bf = const.tile([P, P], bf)
        nc.vector.tensor_copy(out=ident_bf[:], in_=ident[:])

        idxt = []
        slots = []
        for kk in range(NCLS):
            it = const.tile([128, T_total[kk] * 8], mybir.dt.int16, tag=f"idx{kk}")
            nc.sync.dma_start(out=it[:], in_=idx_p[kk][:, :])
            idxt.append(it)
            st = const.tile([P, T_total[kk]], bf, tag=f"slot{kk}")
            nc.sync.dma_start(out=st[:], in_=slot_p[kk][:, :])
            slots.append(st)

        degt = const.tile([P, B], f32)
        nc.sync.dma_start(out=degt[:], in_=deg_p[:, :])
        sdeg = const.tile([P, B], f32)
        nc.scalar.activation(out=sdeg[:], in_=degt[:],
                             func=mybir.ActivationFunctionType.Sqrt)
        dinv = const.tile([P, B], f32)
        nc.vector.reciprocal(out=dinv[:], in_=sdeg[:])

        # ---- phase 1: g1 = dinv * (x @ W1), class-split AllGather ----
        # activations land in per-class staging tiles; ONE DMA ships each
        # class to DRAM (cuts 49 per-block DMA dispatches off the start)
        stage = []
        for kk in range(NCLS):
            stg = const.tile([P, CLS_BLK[kk], D], bf, tag=f"stage{kk}")
            stage.append(stg)
        # class-1 (17 blocks) is computed FIRST so its (smaller) AllGather
        # completes early and the layer-1 class-1 gathers can start ~60us
        # sooner; class-0 follows while those gathers run.
        with nc.named_scope("phase1"):
            for b in list(range(CLS_BLK[0], B)) + list(range(CLS_BLK[0])):
                cs = slice(b * P, (b + 1) * P)
                ph = psum.tile([P, D], f32, tag="ph")
                nc.tensor.matmul(out=ph[:], lhsT=xT[:, cs], rhs=w1[:],
                                 start=True, stop=True)
                kb = 0 if b < CLS_BLK[0] else 1
                bo = b - (0 if kb == 0 else CLS_BLK[0])
                nc.scalar.activation(out=stage[kb][:, bo, :], in_=ph[:],
                                     func=mybir.ActivationFunctionType.Copy,
                                     scale=dinv[:, b:b + 1])
                if b == B - 1:
                    nc.sync.dma_start(
                        out=g_own[0][1].reshape([CLS_BLK[1], P, D])
                            .transpose([1, 0, 2])[:, :, :],
                        in_=stage[1][:])
                    with nc.named_scope("ag1b"):
                        nc.gpsimd.collective_compute(
                            "AllGather", mybir.AluOpType.bypass,
                            replica_groups=[list(range(C))],
                            ins=[g_own[0][1][:, :]], outs=[g_full[0][1][:, :]])
            nc.sync.dma_start(
                out=g_own[0][0].reshape([CLS_BLK[0], P, D])
                    .transpose([1, 0, 2])[:, :, :],
                in_=stage[0][:])
            with nc.named_scope("ag1a"):
                nc.gpsimd.collective_compute(
                    "AllGather", mybir.AluOpType.bypass,
                    replica_groups=[list(range(C))],
                    ins=[g_own[0][0][:, :]], outs=[g_full[0][0][:, :]])

        def emit_gather(l, kk, ch):
            nt = ch['t1'][kk] - ch['t0'][kk]
            if nt == 0:
                return None
            gt = gbuf.tile([P, max_ct[kk], D], bf, tag=f"g{kk}")
            nc.gpsimd.dma_gather(
                out_ap=gt[:, :nt, :], in_ap=g_full[l][kk][:, :],
                idxs_ap=idxt[kk][:, ch['t0'][kk] * 8:ch['t1'][kk] * 8],
                num_idxs=nt * P, num_idxs_reg=nt * P, elem_size=D,
                single_packet=False)
            return gt

        def emit_block(l, b, ch, gts):
            cs = slice(b * P, (b + 1) * P)
            pB = psum.tile([P, D], f32, tag="pB")
            nc.tensor.matmul(out=pB[:], lhsT=ones1[:], rhs=bcomb[l][:],
                             start=True, stop=False)
            if l == 0:
                nc.tensor.matmul(out=pB[:], lhsT=xT[:, cs], rhs=rwf[:],
                                 start=False, stop=True)
            else:
                nc.tensor.matmul(out=pB[:], lhsT=x1T[:, cs], rhs=rwb[:],
                                 start=False, stop=True)
            pA = psum.tile([P, D], f32, tag="pA")
            # self-loop: psum += I @ g_own[block rows]
            kb = 0 if b < CLS_BLK[0] else 1
            lo = b * P - CLS_BASE[kb]
            gsb = work.tile([P, D], bf, tag="gsb")
            nc.sync.dma_start(out=gsb[:], in_=g_own[l][kb][lo:lo + P, :])
            ntot = T[0][b] + T[1][b]
            nc.tensor.matmul(out=pA[:], lhsT=ident_bf[:], rhs=gsb[:],
                             start=True, stop=(ntot == 0))
            done = 0
            for kk in range(NCLS):
                ntk = T[kk][b]
                if ntk == 0:
                    continue
                tb = tile_base[kk][b]
                toff = tb - ch['t0'][kk]
                selb = work.tile([P, selmax[kk], P], bf, tag=f"sel{kk}")
                nc.vector.tensor_tensor(
                    out=selb[:, :ntk, :],
                    in0=slots[kk][:, tb:tb + ntk]
                        .rearrange("p (k o) -> p k o", o=1)
                        .to_broadcast([P, ntk, P]),
                    in1=iota[:].rearrange("p (o d) -> p o d", o=1)
                        .to_broadcast([P, ntk, P]),
                    op=mybir.AluOpType.is_equal)
                for t in range(ntk):
                    done += 1
                    nc.tensor.matmul(out=pA[:], lhsT=selb[:, t, :],
                                     rhs=gts[kk][:, toff + t, :],
                                     start=False, stop=(done == ntot))
            t1 = outp.tile([P, D], f32, tag="t1")
            nc.scalar.activation(out=t1[:], in_=pA[:],
                                 func=mybir.ActivationFunctionType.Copy,
                                 scale=dinv[:, b:b + 1])
            t2 = outp.tile([P, D], f32, tag="t2")
            nc.vector.tensor_tensor(out=t2[:], in0=t1[:], in1=pB[:],
                                    op=mybir.AluOpType.add)
            xo = outp.tile([P, D], f32, tag="xo")
            nc.scalar.activation(out=xo[:], in_=t2[:],
                                 func=mybir.ActivationFunctionType.Relu)
            if l == 0:
                pT = psum.tile([P, D], f32, tag="pT")
                nc.tensor.transpose(out=pT[:], in_=xo[:], identity=ident[:])
                nc.vector.tensor_copy(out=x1T[:, cs], in_=pT[:])
                ph2 = psum.tile([P, D], f32, tag="ph")
                nc.tensor.matmul(out=ph2[:], lhsT=x1T[:, cs], rhs=w2b[:],
                                 start=True, stop=True)
                g2b = outp.tile([P, D], bf, tag="gb")
                nc.scalar.activation(out=g2b[:], in_=ph2[:],
                                     func=mybir.ActivationFunctionType.Copy,
                                     scale=dinv[:, b:b + 1])
                nc.sync.dma_start(out=g_own[1][kb][lo:lo + P, :], in_=g2b[:])
            else:
                nc.sync.dma_start(out=out_p[cs, :], in_=xo[:])

        # ---- layer 1 ----
        with nc.named_scope("layer1"):
            for ci, ch in enumerate(chunks):
                gt1 = emit_gather(0, 1, ch)
                gts = [emit_gather(0, 0, ch), gt1]
                if ci == 8:
                    # class-0 g2 rows (blocks 0..31) are written by now;
                    # the collective runs while the last chunks' gathers go
                    with nc.named_scope("ag2a"):
                        nc.gpsimd.collective_compute(
                            "AllGather", mybir.AluOpType.bypass,
                            replica_groups=[list(range(C))],
                            ins=[g_own[1][0][:, :]], outs=[g_full[1][0][:, :]])
                for b in range(ch['b0'], ch['b1']):
                    emit_block(0, b, ch, gts)
        # ---- layer 2 (class-1 AllGather slotted after the first gather) ----
        with nc.named_scope("layer2"):
            for ci, ch in enumerate(chunks):
                gts = [emit_gather(1, 0, ch)]
                if ci == 0:
                    with nc.named_scope("ag2b"):
                        nc.gpsimd.collective_compute(
                            "AllGather", mybir.AluOpType.bypass,
                            replica_groups=[list(range(C))],
                            ins=[g_own[1][1][:, :]], outs=[g_full[1][1][:, :]])
                gts.append(emit_gather(1, 1, ch))
                for b in range(ch['b0'], ch['b1']):
                    emit_block(1, b, ch, gts)
    return nc


# ---------------------------------------------------------------------------
# Entry point
# ---------------------------------------------------------------------------

def make_inputs(x, conv_w, conv_b, res_w, res_b, meta):
    npc, npad = meta['npc'], meta['npad']
    iota = np.tile(np.arange(P, dtype=np.float32), (P, 1)).astype(bf16)
    warm = np.zeros((128, 8), dtype=np.int16)
    in_maps = []
    for c in range(C):
        xT = np.zeros((P, npad), dtype=np.float32)
        xT[:, :npc] = np.asarray(x[c * npc:(c + 1) * npc], dtype=np.float32).T
        in_maps.append({
            "xT": xT,
            "w1": np.asarray(conv_w[0], dtype=np.float32),
            "w2": np.asarray(conv_w[1], dtype=np.float32),
            "resw": np.asarray(res_w, dtype=np.float32),
            "convb": np.asarray(conv_b, dtype=np.float32),
            "resb": np.asarray(res_b, dtype=np.float32).reshape(1, D),
            "deg": meta['deg'][c],
            "idx0": meta['idx'][0][c],
            "idx1": meta['idx'][1][c],
            "slot0": meta['slots'][0][c].astype(bf16),
            "slot1": meta['slots'][1][c].astype(bf16),
            "warmidx": warm,
            "iota": iota,
            "ident": np.eye(P, dtype=np.float32),
            "ones": np.ones((1, D), dtype=np.float32).astype(bf16),
        })
    return in_maps


def run(x, edge_index, conv_w, conv_b, res_w, res_b, trace=False, trace_kwargs=None):
    N = x.shape[0]
    meta = prep(edge_index, N)
    nc = build_program(meta)
    nc.compile()
    split_sync_waits(nc)
    in_maps = make_inputs(x, conv_w, conv_b, res_w, res_b, meta)
    res = run_bass_kernel_spmd(nc, in_maps, list(range(C)), trace=trace,
                               **(trace_kwargs or {}))
    npc = meta['npc']
    out = np.concatenate([np.asarray(res.results[c]["out"])[:npc]
                          for c in range(C)], axis=0)
    return out.astype(np.float32), res


def kernel(x, edge_index, conv_w, conv_b, res_w, res_b):
    out, _ = run(x, edge_index, conv_w, conv_b, res_w, res_b, trace=False)
    return out
